# revision 56
# baseline (speedup 1.0000x reference)
"""Trainium2 Bass kernel for a causal self-attention block (GQA + gated value
embedding + RoPE + QK-RMSNorm), sharded over 8 NeuronCores.

Sharding: 8 cores = 2 (batch) x 4 (kv-head groups).  Each core computes, for
its batch b and head-group g (4 q-heads + 1 kv-head):
    q/k/v projections, gated ve addition, RoPE, RMSNorm, causal attention,
    and the partial output projection  y_g @ Wproj[g*512:(g+1)*512, :].
The host sums the 4 per-group partials for each batch (the Wproj
contraction distributes over head groups).

v3: bf16 matmuls; host-side x transpose; DMA (XBAR) transposes for qT/kT;
attention in 256-col i-chunks with quad-head S tiles + one exp per k-block;
softmax denominator via ones-matrix broadcast-accumulate matmuls on the PE;
DVE reciprocal finish; single Exp ACT table for the whole kernel; fp16 out.

Self-contained: hardcodes shapes; accepts FULL inputs, returns FULL output.
"""

from contextlib import ExitStack

import numpy as np
import ml_dtypes

import concourse.bacc as bacc
import concourse.bass as bass
import concourse.mybir as mybir
import concourse.tile as tile
from concourse.bass_utils import run_bass_kernel_spmd
from concourse.masks import make_identity

F32 = mybir.dt.float32
F32R = mybir.dt.float32r
BF16 = mybir.dt.bfloat16
FP16 = mybir.dt.float16
I32 = mybir.dt.int32
AF = mybir.ActivationFunctionType
ALU = mybir.AluOpType
AX = mybir.AxisListType

B, C, HD, NHL, GC = 2, 2048, 128, 4, 32  # NHL = local q heads per core
EPS = float(np.finfo(np.float32).eps)
ISQ = 1.0 / float(np.sqrt(128.0))
RSQRT_MAGIC = 0x5F3759DF


def _bcast(ap_, idx, count):
    """Insert a step-0 (broadcast) dim at position idx of the AP dims."""
    lst = [list(p) for p in ap_.ap]
    lst.insert(idx, [0, count])
    return bass.AP(ap_.tensor, ap_.offset, lst)


def build(T=2048):
    TB = T // 128    # token blocks
    CT = C // 128    # contraction tiles for qkv
    NCH = T // 256   # i-chunks for attention (256 wide)
    OC = C // 512    # output chunks for proj
    NG = TB // 4     # 4-block groups in phase A

    nc = bacc.Bacc("TRN2", target_bir_lowering=False, debug=False)
    xt = nc.dram_tensor("xt", [C, T], BF16, kind="ExternalInput")
    wqkv = nc.dram_tensor("wqkv", [C, 768], BF16, kind="ExternalInput")
    wproj = nc.dram_tensor("wproj", [NHL * HD, C], BF16, kind="ExternalInput")
    wg = nc.dram_tensor("wg", [GC, 1], BF16, kind="ExternalInput")
    ve2 = nc.dram_tensor("ve2", [T, HD], BF16, kind="ExternalInput")
    cosn = nc.dram_tensor("cosn", [T, 64], BF16, kind="ExternalInput")
    sinn = nc.dram_tensor("sinn", [T, 64], BF16, kind="ExternalInput")
    out = nc.dram_tensor("out", [T, C], FP16, kind="ExternalOutput")

    with ExitStack() as stk:
        tc = stk.enter_context(tile.TileContext(nc))
        gpool = stk.enter_context(tc.tile_pool(name="gconst", bufs=1))
        ident = gpool.tile([128, 128], F32)
        make_identity(nc, ident)
        identr = gpool.tile([128, 128], F32R)
        nc.vector.tensor_copy(out=identr, in_=ident)
        ones_b = gpool.tile([128, 128], BF16)
        nc.vector.memset(ones_b, 1.0)
        identb = gpool.tile([128, 128], BF16)
        nc.vector.tensor_copy(out=identb, in_=ident)

        persist = stk.enter_context(tc.tile_pool(name="persist", bufs=1))
        # per-group tiles so phase B's deps are on the group, not the last write
        qTg = [persist.tile([128, NHL, 512], BF16, name=f"qT{g}")
               for g in range(T // 512)]          # [d, h, t%512]
        kTg = [persist.tile([128, 512], BF16, name=f"kT{g}")
               for g in range(T // 512)]          # [d, t%512]
        vSg = [persist.tile([128, 4, HD], BF16, name=f"vS{g}")
               for g in range(T // 512)]          # [t%128, tb%4, d]
        yT = persist.tile([128, NHL, T], BF16)   # [d, h, t]
        # rope pool outlives phase A: g3's QH is consumed mid-phase-B
        ropeP = stk.enter_context(tc.tile_pool(name="ropeP", bufs=2))

        # ---------------- weight / input DMAs (granular, ordered) ----------
        wA = stk.enter_context(tc.tile_pool(name="wA", bufs=1))
        wqkv_sb = wA.tile([128, CT, 768], BF16)
        wqkvr = wqkv.rearrange("(ct p) j -> p ct j", p=128)
        xtr = xt.rearrange("(ct p) t -> p ct t", p=128)
        # DMA issue only on sync/gpsimd: a DIRECT2D issue can block its queue
        # for >15us waiting on ring credits, and scalar/vector have phase-A
        # compute (PSUM copies) that must not sit behind that.

        # ---------------- phase A: qkv + rope + rmsnorm + transposes --------
        # transposes live outside phase A (g3's are emitted mid-phase-B)
        pend = []  # (QH, g) awaiting DMA transposes

        def emit_transposes(QH, g, pspool, tag="warm"):
            for tb4 in range(4):
                t0 = tb4 * 128
                for hh in range(5):
                    t_ps = pspool.tile([128, 128], BF16, tag=tag,
                                       padded_shape=[128, 256], name="tps",
                                       bufs=2)
                    nc.tensor.transpose(t_ps, QH[:, tb4, hh, :], identb)
                    dst = (qTg[g][:, hh, t0:t0 + 128] if hh < 4
                           else kTg[g][:, t0:t0 + 128])
                    if (tb4 * 5 + hh) % 2 == 0:
                        nc.vector.tensor_copy(out=dst, in_=t_ps)
                    else:
                        nc.scalar.copy(out=dst, in_=t_ps)

        with nc.named_scope("phaseA"), \
                tc.tile_pool(name="xTp", bufs=2) as xTp, \
                tc.tile_pool(name="psA", bufs=2, space="PSUM") as psA, \
                tc.tile_pool(name="psg", bufs=1, space="PSUM") as psg, \
                tc.tile_pool(name="qkvP", bufs=2) as qkvP, \
                tc.tile_pool(name="smallP", bufs=2) as smallP:

            # prefetch x-transpose superblocks (gpsimd queue)
            def load_xT(g):
                xT_t = xTp.tile([128, CT, 512], BF16, tag="xT", name="xT_t")
                nc.gpsimd.dma_start(
                    out=xT_t, in_=xtr[:, :, g * 512:(g + 1) * 512])
                return xT_t

            # monolithic loads: the DMA rings starve on descriptor feed when
            # loads are chunked (issue ~900ns each + ring-credit blocking), so
            # one big dma per tensor saturates them fastest.  wqkv on gpsimd,
            # xT-g0 + small tables on sync stream in parallel.
            xT0 = xTp.tile([128, CT, 512], BF16, tag="xT", name="xT_t")
            wg_sb = wA.tile([GC, 1], BF16)
            nc.sync.dma_start(out=wg_sb, in_=wg[:, :])  # 64B, gates tb4=0
            nc.gpsimd.dma_start(out=wqkv_sb, in_=wqkvr[:, :, :])
            nc.sync.dma_start(out=xT0, in_=xtr[:, :, 0:512])
            cos_sb = wA.tile([128, TB, 64], BF16)
            nc.sync.dma_start(
                out=cos_sb, in_=cosn.rearrange("(tb p) d -> p tb d", p=128))
            sin_sb = wA.tile([128, TB, 64], BF16)
            nc.sync.dma_start(
                out=sin_sb, in_=sinn.rearrange("(tb p) d -> p tb d", p=128))
            ve_sb = wA.tile([128, TB, HD], BF16)
            nc.sync.dma_start(
                out=ve_sb, in_=ve2.rearrange("(tb p) d -> p tb d", p=128))
            xts = {0: xT0}
            if NG > 1:
                xts[1] = load_xT(1)
            wp_sb = wA.tile([128, NHL, OC, 512], BF16)  # dma emitted at g==1

            # PE warmup: junk matmuls keep the PE clocked up (pstate ramps on
            # continuous execution) while the first ~5MB of DMAs land.
            junk = wA.tile([128, 512], BF16)
            nc.vector.memset(junk, 1.0)
            for _ in range(16):
                w_ps = psA.tile([128, 512], F32, tag="warm", name="wps",
                                bufs=2)
                nc.tensor.matmul(w_ps, lhsT=ones_b, rhs=junk,
                                 start=True, stop=True)

            for g in range(NG):
                if g == 1:  # wproj only needed in phase C; don't steal BW early
                    nc.gpsimd.dma_start(
                        out=wp_sb,
                        in_=wproj.rearrange("(h p) (oc o) -> p h oc o",
                                            p=128, o=512))
                if g + 1 < NG and g + 1 not in xts:
                    xts[g + 1] = load_xT(g + 1)
                xT_t = xts.pop(g)
                qkvS = qkvP.tile([128, 4, 768], BF16, tag="qkvS")
                zg_ps = psg.tile([128, 4], F32, tag="zg")
                for tb4 in range(4):
                    q_ps = psA.tile([128, 512], F32, tag="q")
                    kv_ps = psA.tile([128, 256], F32, tag="kv",
                                     padded_shape=[128, 512])
                    for ct in range(CT):
                        lhsT = xT_t[:, ct, tb4 * 128:(tb4 + 1) * 128]
                        nc.tensor.matmul(
                            q_ps, lhsT=lhsT, rhs=wqkv_sb[:, ct, 0:512],
                            start=(ct == 0), stop=(ct == CT - 1))
                        nc.tensor.matmul(
                            kv_ps, lhsT=lhsT, rhs=wqkv_sb[:, ct, 512:768],
                            start=(ct == 0), stop=(ct == CT - 1))
                    # gate logits z = x[:, :GC] @ wgate (x cols 0..31 = xT rows)
                    nc.tensor.matmul(
                        zg_ps[:, tb4:tb4 + 1],
                        lhsT=xT_t[0:GC, 0, tb4 * 128:(tb4 + 1) * 128],
                        rhs=wg_sb, start=True, stop=True)
                    nc.scalar.copy(out=qkvS[:, tb4, 0:512], in_=q_ps)
                    nc.scalar.copy(out=qkvS[:, tb4, 512:768], in_=kv_ps)
                    if tb4 == 1 and pend:
                        emit_transposes(*pend.pop(0), psA)

                # ---- gate = sigmoid(z) via Exp table + DVE ----
                e_sb = smallP.tile([128, 4], F32, tag="esb")
                nc.scalar.activation(e_sb, zg_ps, AF.Exp, scale=-1.0)
                nc.vector.tensor_scalar_add(e_sb, e_sb, 1.0)
                g_sb = smallP.tile([128, 4], F32, tag="gsb")
                nc.vector.reciprocal_approx_fast(g_sb, e_sb)
                for tb4 in range(4):
                    tb = 4 * g + tb4
                    nc.vector.scalar_tensor_tensor(
                        out=vSg[g][:, tb4, :], in0=ve_sb[:, tb, :],
                        scalar=g_sb[:, tb4:tb4 + 1], in1=qkvS[:, tb4, 640:768],
                        op0=ALU.mult, op1=ALU.add)

                # ---- batched rope over 4 tb x 5 heads (bf16, 2x DVE mode) ----
                qk = qkvS[:, :, 0:640].rearrange("p tb (h d) -> p tb h d", h=5)
                QR = ropeP.tile([128, 4, 5, 128], BF16, tag="QR", bufs=1)
                tmp = ropeP.tile([128, 4, 5, 64], BF16, tag="tmp", bufs=1)
                cosB = _bcast(cos_sb[:, 4 * g:4 * g + 4, :], 2, 5)
                sinB = _bcast(sin_sb[:, 4 * g:4 * g + 4, :], 2, 5)
                q1 = qk[:, :, :, 0:64]
                q2 = qk[:, :, :, 64:128]
                r1 = QR[:, :, :, 0:64]
                r2 = QR[:, :, :, 64:128]
                nc.vector.tensor_tensor(out=r1, in0=q1, in1=cosB, op=ALU.mult)
                nc.vector.tensor_tensor(out=tmp, in0=q2, in1=sinB, op=ALU.mult)
                nc.vector.tensor_tensor(out=r1, in0=r1, in1=tmp, op=ALU.add)
                nc.vector.tensor_tensor(out=r2, in0=q2, in1=cosB, op=ALU.mult)
                nc.vector.tensor_tensor(out=tmp, in0=q1, in1=sinB, op=ALU.mult)
                nc.vector.tensor_tensor(out=r2, in0=r2, in1=tmp, op=ALU.subtract)

                # ---- rms scales: mean of squares over d, newton rsqrt ----
                sqt = ropeP.tile([128, 4, 5, 128], BF16, tag="sqt", bufs=1)
                nc.vector.tensor_tensor(out=sqt, in0=QR, in1=QR, op=ALU.mult)
                red = smallP.tile([128, 4, 5], F32, tag="red")
                nc.vector.tensor_reduce(out=red, in_=sqt, axis=AX.X, op=ALU.add)
                nc.vector.tensor_scalar(
                    out=red, in0=red, scalar1=1.0 / 128.0, scalar2=EPS,
                    op0=ALU.mult, op1=ALU.add)
                rq = smallP.tile([128, 4, 5], F32, tag="rq")
                rqi = rq.bitcast(I32)
                nc.vector.tensor_scalar(
                    out=rqi, in0=red.bitcast(I32), scalar1=1, scalar2=None,
                    op0=ALU.logical_shift_right)
                nc.vector.tensor_scalar(
                    out=rqi, in0=rqi, scalar1=-1, scalar2=RSQRT_MAGIC,
                    op0=ALU.mult, op1=ALU.add)
                nt = smallP.tile([128, 4, 5], F32, tag="nt")
                for _ in range(2):
                    nc.vector.tensor_tensor(out=nt, in0=rq, in1=rq, op=ALU.mult)
                    nc.vector.tensor_tensor(out=nt, in0=nt, in1=red, op=ALU.mult)
                    nc.vector.tensor_scalar(
                        out=nt, in0=nt, scalar1=-0.5, scalar2=1.5,
                        op0=ALU.mult, op1=ALU.add)
                    nc.vector.tensor_tensor(out=rq, in0=rq, in1=nt, op=ALU.mult)
                QH = ropeP.tile([128, 4, 5, 128], BF16, tag="QH", bufs=4)
                nc.vector.tensor_tensor(
                    out=QH, in0=QR, in1=_bcast(rq, 3, 128), op=ALU.mult)

                pend.append((QH, g))
            # NOTE: g3 stays in `pend`; its transposes are emitted mid-phase-B
            # so the PE chews on attention chunks while DVE finishes g3's rope.

        # ---------------- phase B: attention ----------------
        # 128-col i-chunks; all 4 heads in ONE matmul per k-block for each of
        # S / AV / denominator (N=512, one PSUM bank each -> 6 banks total).
        # Chunks 11..0 run first (they only need head-groups 0-2), then g3's
        # transposes, then chunks 12..15; phase C's PSUM pool uses the 2 free
        # banks so the projection overlaps phase B's drain.
        NCH8 = T // 128
        with nc.named_scope("phaseB"), \
                tc.tile_pool(name="spsP", bufs=2, space="PSUM") as spsP, \
                tc.tile_pool(name="psy", bufs=2, space="PSUM") as psy, \
                tc.tile_pool(name="psd", bufs=2, space="PSUM") as psd, \
                tc.tile_pool(name="ptP", bufs=8) as ptP, \
                tc.tile_pool(name="accP", bufs=3) as accP, \
                tc.tile_pool(name="finP", bufs=2) as finP, \
                tc.tile_pool(name="sbC", bufs=6) as sbC, \
                tc.tile_pool(name="psC", bufs=2, space="PSUM") as psC:

            avq = []  # pending AV matmuls, drained at lag 6
            dq = []   # pending denominator group matmuls, drained at lag 1
            fin = {}  # c -> [av_pending, d_pending]

            # phase-C projection, emitted in per-oc pieces: woven into the
            # exp-bound attention stream (PE has ~140ns/k-block of slack
            # there), the rest as a straight tail.  Woven copies go to the
            # mostly-idle GpSimd engine; tail copies to scalar/vector.
            cpieces = []
            cstate = {"osb": None, "cnt": 0}

            def push_tb(tb):
                for oc in range(OC):
                    cpieces.append((tb, oc))

            def emit_cpiece(tail):
                tb, oc = cpieces.pop(0)
                t0 = tb * 128
                if oc == 0:
                    cstate["osb"] = sbC.tile([128, C], FP16, tag="osb",
                                             name="osb")
                o_sb = cstate["osb"]
                o_ps = psC.tile([128, 512], F32, tag="ops", name="o_ps")
                for hh in range(NHL):
                    nc.tensor.matmul(
                        o_ps, lhsT=yT[:, hh, t0:t0 + 128],
                        rhs=wp_sb[:, hh, oc, :],
                        start=(hh == 0), stop=(hh == NHL - 1))
                dst = o_sb[:, oc * 512:(oc + 1) * 512]
                cstate["cnt"] += 1
                if not tail:
                    # woven piece: copy on vector (scalar is the exp-bound
                    # engine in this window; gpsimd/DMA can't touch PSUM)
                    nc.vector.tensor_copy(out=dst, in_=o_ps)
                elif cstate["cnt"] % 2 == 0:
                    nc.scalar.copy(out=dst, in_=o_ps)
                else:
                    nc.vector.tensor_copy(out=dst, in_=o_ps)
                if oc % 2 == 1:
                    eng = nc.sync if cstate["cnt"] % 4 < 2 else nc.gpsimd
                    nc.void = eng.dma_start(
                        out=out[t0:t0 + 128, (oc - 1) * 512:(oc + 1) * 512],
                        in_=o_sb[:, (oc - 1) * 512:(oc + 1) * 512])

            def finalize(c, yps, bda):
                i0 = c * 128
                rcp = finP.tile([128, 4, 128], F32, tag="rcp")
                nc.vector.reciprocal_approx_fast(rcp, bda)
                nc.vector.tensor_tensor(
                    out=yT[:, :, i0:i0 + 128], in0=yps, in1=rcp, op=ALU.mult)

            def emit_AV(pt, jb, yps, bda, last, c):
                nc.tensor.matmul(
                    yps, lhsT=vSg[jb // 4][:, jb % 4, :], rhs=pt,
                    start=(jb == 0), stop=last)
                if last:
                    st = fin[c]
                    st[0] = False
                    if not st[1]:
                        finalize(c, yps, bda)

            def emit_D(dsrc, gi, last, yps, bda, c):
                # denominator: one broadcast-accumulate matmul per <=4-block
                # group (the group was pre-summed on DVE), not one per block
                nc.tensor.matmul(
                    bda, lhsT=ones_b, rhs=dsrc, start=(gi == 0), stop=last)
                if last:
                    st = fin[c]
                    st[1] = False
                    if not st[0]:
                        finalize(c, yps, bda)

            def do_chunk(c):
                qg, qo = c // 4, (c % 4) * 128
                yps = psy.tile([128, 4, 128], F32, tag="yps", name="yps")
                bda = psd.tile([128, 4, 128], F32, tag="bda", name="bda")
                fin[c] = [True, True]
                ngrp = (c + 4) // 4
                grp = []
                gi = 0
                for jb in range(c + 1):
                    sps = spsP.tile([128, 4, 128], F32, tag="s")
                    nc.tensor.matmul(
                        sps,
                        lhsT=kTg[jb // 4][:, (jb % 4) * 128:(jb % 4 + 1) * 128],
                        rhs=qTg[qg][:, :, qo:qo + 128],
                        start=True, stop=True)
                    pt = ptP.tile([128, 4, 128], BF16, tag="pt")
                    nc.scalar.activation(pt, sps, AF.Exp, scale=ISQ)
                    if jb == c:  # diagonal block: zero j > i
                        for h in range(NHL):
                            nc.gpsimd.affine_select(
                                out=pt[:, h, :], in_=pt[:, h, :],
                                pattern=[[1, 128]], compare_op=ALU.is_ge,
                                fill=0.0, base=0, channel_multiplier=-1)
                    grp.append(pt)
                    if len(grp) == 4 or jb == c:
                        if len(grp) == 1:
                            dsrc = grp[0]
                        else:
                            acc = accP.tile([128, 4, 128], BF16, tag="pacc",
                                            name="pacc")
                            nc.vector.tensor_tensor(
                                out=acc, in0=grp[0], in1=grp[1], op=ALU.add)
                            for t2 in grp[2:]:
                                nc.vector.tensor_tensor(
                                    out=acc, in0=acc, in1=t2, op=ALU.add)
                            dsrc = acc
                        dq.append((dsrc, gi, gi == ngrp - 1, yps, bda, c))
                        grp = []
                        gi += 1
                    avq.append((pt, jb, yps, bda, jb == c, c))
                    if len(avq) > 6:
                        emit_AV(*avq.pop(0))
                    if len(dq) > 1:
                        emit_D(*dq.pop(0))
                    if jb % 5 == 3 and cpieces:
                        emit_cpiece(tail=False)

            def drain():
                while avq or dq:
                    if avq:
                        emit_AV(*avq.pop(0))
                    if dq:
                        emit_D(*dq.pop(0))

            for c in range(11, -1, -1):
                do_chunk(c)
                if c <= 9:
                    push_tb(c + 2)   # chunk c+2 finalized >=1 chunk ago
            drain()
            for tb in (2, 1, 0):
                push_tb(tb)
            # ------------ phase B tail + phase C straight tail -------------
            with nc.named_scope("phaseC"):
                # g3 transposes: PE picks these up while DVE finishes g3 rope
                while pend:
                    emit_transposes(*pend.pop(0), psC, tag="ops")
                for c in range(12, NCH8):
                    do_chunk(c)
                drain()
                for tb in (12, 13, 14, 15):
                    push_tb(tb)
                while cpieces:
                    emit_cpiece(tail=True)

    nc.compile()
    return nc


_NC_CACHE = {}


def get_nc(T=2048):
    if T not in _NC_CACHE:
        _NC_CACHE[T] = build(T)
    return _NC_CACHE[T]


def make_in_maps(x, ve, cos, sin, Wq, Wk, Wv, Wproj, Wgate):
    """Shard full inputs into 8 per-core input maps (2 batch x 4 head groups)."""
    bf16 = ml_dtypes.bfloat16
    x = np.asarray(x, np.float32)
    ve = np.asarray(ve, np.float32)
    cosn = np.ascontiguousarray(np.asarray(cos, np.float32)[0, :, 0, :])
    sinn = np.ascontiguousarray(np.asarray(sin, np.float32)[0, :, 0, :])
    Wq = np.asarray(Wq, np.float32)
    Wk = np.asarray(Wk, np.float32)
    Wv = np.asarray(Wv, np.float32)
    Wproj = np.asarray(Wproj, np.float32)
    Wgate = np.asarray(Wgate, np.float32)
    xts = [np.ascontiguousarray(x[b].T).astype(bf16) for b in range(B)]
    in_maps = []
    for core in range(8):
        b, g = divmod(core, 4)
        wqkv = np.concatenate(
            [Wq[:, g * 512:(g + 1) * 512],
             Wk[:, g * 128:(g + 1) * 128],
             Wv[:, g * 128:(g + 1) * 128]], axis=1).astype(bf16)
        in_maps.append({
            "xt": xts[b],
            "wqkv": np.ascontiguousarray(wqkv),
            "wproj": np.ascontiguousarray(
                Wproj[g * 512:(g + 1) * 512, :]).astype(bf16),
            "wg": np.ascontiguousarray(Wgate[:, g:g + 1]).astype(bf16),
            "ve2": np.ascontiguousarray(
                2.0 * ve[b][:, g * 128:(g + 1) * 128]).astype(bf16),
            "cosn": cosn.astype(bf16),
            "sinn": sinn.astype(bf16),
        })
    return in_maps


def run_cores(in_maps, trace=False, **kw):
    nc = get_nc(in_maps[0]["xt"].shape[1])
    return run_bass_kernel_spmd(nc, in_maps, core_ids=list(range(8)), trace=trace, **kw)


def kernel(**inputs):
    in_maps = make_in_maps(
        inputs["x"], inputs["ve"], inputs["cos"], inputs["sin"],
        inputs["Wq"], inputs["Wk"], inputs["Wv"], inputs["Wproj"], inputs["Wgate"])
    res = run_cores(in_maps)
    parts = [np.asarray(res.results[i]["out"], np.float32) for i in range(8)]
    out = np.stack([
        parts[0] + parts[1] + parts[2] + parts[3],
        parts[4] + parts[5] + parts[6] + parts[7],
    ]).astype(np.float32)
    return out

